# revision 8
# baseline (speedup 1.0000x reference)
"""Trainium2 Bass kernel: 3-layer GAT + BN + ELU + residual + global mean pool + linear.

Sharding: nodes (and their incident edges, grouped by destination) are
sharded across 8 NeuronCores. Weights replicated. Per layer:
  1. local h_ext = x_local @ [W | W@As | W@Ad]  (node-major rows)
  2. AllGather h_ext -> full [N, ROW] table in DRAM (bf16)
  3. per dst-block: dma_gather of h_ext[src] rows for this core's edges,
     attention weights w = exp(leaky(sS[src]+sD[dst])) via one-hot
     broadcast matmul, weighted scatter-matmul into PSUM (U), Z by
     masked reduce, y = U/(Z+eps)
  4. BN stats (ones-matmul) -> AllReduce -> scale/shift -> ELU -> residual
Pool + final linear at the end (AllReduce of pooled sums).
"""
import sys
if '/opt/trn_rl_repo' not in sys.path:
    sys.path.insert(0, '/opt/trn_rl_repo')
import numpy as np
import ml_dtypes

import concourse.bass as bass
import concourse.bacc as bacc
import concourse.mybir as mybir
from concourse import tile
from concourse.bass_utils import run_bass_kernel_spmd

F32 = mybir.dt.float32
BF16 = mybir.dt.bfloat16
I16 = mybir.dt.int16
AL = mybir.AluOpType
ACTF = mybir.ActivationFunctionType
AX = mybir.AxisListType

N, E, FIN, H, C, G, NCLS = 10000, 160000, 512, 8, 64, 64, 64
P = 8
NL = N // P            # 1250 nodes per core
NT = 10                # node tiles per core (9x128 + 98)
LAST = NL - 9 * 128    # 98
CH = 6                 # gather-chunk size in 128-edge slots
ROW12 = 640            # bf16 gather row (640*2B = 1280B, %256==0); data in 0:528
ROW3 = 128             # bf16 gather row L3 (256B); data in 0:66
SS12 = 528             # h(512) | sS(8) | sD(8)
SS3 = 66               # h(64) | sS(1) | sD(1)
EPS_Z = 1e-16
EPS_BN = 1e-5
NP_BF16 = ml_dtypes.bfloat16


def _blockdiag(a):
    # a [H, C] -> [H*C, H] with column h holding a[h] in rows h*C:(h+1)*C
    hh, cc = a.shape
    out = np.zeros((hh * cc, hh), np.float64)
    for h in range(hh):
        out[h * cc:(h + 1) * cc, h] = a[h]
    return out


def _prep(inputs):
    x = np.asarray(inputs['x'], np.float32)
    ei = np.asarray(inputs['edge_index'], np.int64)
    batch = np.asarray(inputs['batch'], np.int64)

    src = np.concatenate([ei[0], np.arange(N, dtype=np.int64)])
    dst = np.concatenate([ei[1], np.arange(N, dtype=np.int64)])
    order = np.argsort(dst, kind='stable')
    src, dst = src[order], dst[order]

    core = dst // NL
    blk = (dst % NL) // 128
    dloc = (dst % NL) % 128

    per_cb = {}
    T = np.ones(NT, np.int64)
    for c in range(P):
        m = core == c
        sc, dc, bc = src[m], dloc[m], blk[m]
        for b in range(NT):
            mb = bc == b
            per_cb[(c, b)] = (sc[mb], dc[mb])
            T[b] = max(T[b], (int(mb.sum()) + 127) // 128)
    sbase = np.zeros(NT, np.int64)
    sbase[1:] = np.cumsum(T)[:-1]
    TT = int(T.sum())
    NE = TT * 128

    per_core = []
    for c in range(P):
        sidx = np.zeros(NE, np.int64)
        dl = np.full(NE, 255, np.int64)
        for b in range(NT):
            sc, dc = per_cb[(c, b)]
            off = int(sbase[b]) * 128
            sidx[off:off + len(sc)] = sc
            dl[off:off + len(dc)] = dc
        j = np.arange(NE)
        t, pp = j // 128, j % 128
        valid = dl < 128
        S = np.zeros((TT, 128, 128), NP_BF16)
        S[t[valid], pp[valid], dl[valid]] = 1
        ST = np.ascontiguousarray(S.transpose(0, 2, 1))
        mask = np.zeros((128, TT), NP_BF16)
        mask[pp[valid], t[valid]] = 1.0
        g16 = np.zeros((16, NE // 16), np.int16)
        g16[j % 16, j // 16] = sidx.astype(np.int16)
        gidx = np.tile(g16, (8, 1))

        xc = x[c * NL:(c + 1) * NL]                      # [1250, 512]
        x0T = np.zeros((FIN, NT * 128), np.float32)
        x0T[:, :NL] = xc.T
        x0T = x0T.astype(NP_BF16)

        cnt = np.bincount(batch, minlength=G).astype(np.float64)
        inv = 1.0 / np.maximum(cnt, 1.0)
        pool = np.zeros((NT, 128, G), np.float32)
        nodes = np.arange(NL) + c * NL
        nn, ppp = np.arange(NL) // 128, np.arange(NL) % 128
        pool[nn, ppp, batch[nodes]] = inv[batch[nodes]]

        per_core.append(dict(S=S, ST=ST, mask=mask, gidx=gidx, x0T=x0T,
                             pool=pool))

    f64 = lambda k: np.asarray(inputs[k], np.float64)
    W1, W2, W3 = f64('W1'), f64('W2'), f64('W3')
    Wcat1 = np.concatenate(
        [W1, W1 @ _blockdiag(f64('as1')), W1 @ _blockdiag(f64('ad1'))], axis=1)
    Wcat2 = np.concatenate(
        [W2, W2 @ _blockdiag(f64('as2')), W2 @ _blockdiag(f64('ad2'))], axis=1)
    Wcat3 = np.concatenate(
        [W3, (W3 @ f64('as3')[0])[:, None], (W3 @ f64('ad3')[0])[:, None]],
        axis=1)
    encW = f64('enc_W')
    RHS0 = np.concatenate([encW, encW @ Wcat1], axis=1)       # [512, 1040]
    eb1 = (f64('enc_b') @ Wcat1)[None, :]                      # [1, 528]

    shared = dict(
        rhs0=RHS0.astype(NP_BF16),
        w2=Wcat2.astype(NP_BF16),
        w3=Wcat3.astype(NP_BF16),
        encb=np.asarray(inputs['enc_b'], np.float32)[None, :],
        eb1=eb1.astype(np.float32),
        g1=np.asarray(inputs['g1'], np.float32)[None, :],
        be1=np.asarray(inputs['be1'], np.float32)[None, :],
        g2=np.asarray(inputs['g2'], np.float32)[None, :],
        be2=np.asarray(inputs['be2'], np.float32)[None, :],
        g3=np.asarray(inputs['g3'], np.float32)[None, :],
        be3=np.asarray(inputs['be3'], np.float32)[None, :],
        linW=np.asarray(inputs['lin_W'], np.float32),
        linb=np.asarray(inputs['lin_b'], np.float32)[:, None],
        ident=np.eye(128, dtype=np.float32),
    )
    return T.tolist(), TT, per_core, shared


def _build(T_list, TT):
    nc = bacc.Bacc(None, target_bir_lowering=False, debug=False, num_devices=P)
    NE = TT * 128
    sbase = [0] * NT
    for b in range(1, NT):
        sbase[b] = sbase[b - 1] + T_list[b - 1]
    TMAXB = max(T_list)

    # ---- external inputs ----
    S_d = nc.dram_tensor("S", [TT, 128, 128], BF16, kind="ExternalInput")
    ST_d = nc.dram_tensor("ST", [TT, 128, 128], BF16, kind="ExternalInput")
    gidx_d = nc.dram_tensor("gidx", [128, NE // 16], I16, kind="ExternalInput")
    x0T_d = nc.dram_tensor("x0T", [FIN, NT * 128], BF16, kind="ExternalInput")
    pool_d = nc.dram_tensor("pool", [NT, 128, G], F32, kind="ExternalInput")
    rhs0_d = nc.dram_tensor("rhs0", [FIN, 1040], BF16, kind="ExternalInput")
    w2_d = nc.dram_tensor("w2", [FIN, SS12], BF16, kind="ExternalInput")
    w3_d = nc.dram_tensor("w3", [FIN, SS3], BF16, kind="ExternalInput")
    encb_d = nc.dram_tensor("encb", [1, FIN], F32, kind="ExternalInput")
    eb1_d = nc.dram_tensor("eb1", [1, SS12], F32, kind="ExternalInput")
    bn_d = {}
    for ly, wd in ((1, FIN), (2, FIN), (3, C)):
        bn_d[ly] = (nc.dram_tensor(f"g{ly}", [1, wd], F32, kind="ExternalInput"),
                    nc.dram_tensor(f"be{ly}", [1, wd], F32, kind="ExternalInput"))
    linW_d = nc.dram_tensor("linW", [C, NCLS], F32, kind="ExternalInput")
    linb_d = nc.dram_tensor("linb", [NCLS, 1], F32, kind="ExternalInput")
    ident_d = nc.dram_tensor("ident", [128, 128], F32, kind="ExternalInput")
    out_d = nc.dram_tensor("out", [G, NCLS], F32, kind="ExternalOutput")

    # ---- internal DRAM ----
    cc_in = {1: nc.dram_tensor("cc_in1", [NL, ROW12], BF16),
             2: nc.dram_tensor("cc_in2", [NL, ROW12], BF16),
             3: nc.dram_tensor("cc_in3", [NL, ROW3], BF16)}
    cc_out = {1: nc.dram_tensor("cc_out1", [N, ROW12], BF16, addr_space="Shared"),
              2: nc.dram_tensor("cc_out2", [N, ROW12], BF16, addr_space="Shared"),
              3: nc.dram_tensor("cc_out3", [N, ROW3], BF16, addr_space="Shared")}
    st_in = {1: nc.dram_tensor("st_in1", [1, 2 * FIN], F32),
             2: nc.dram_tensor("st_in2", [1, 2 * FIN], F32),
             3: nc.dram_tensor("st_in3", [1, 2 * C], F32)}
    st_out = {1: nc.dram_tensor("st_out1", [1, 2 * FIN], F32, addr_space="Shared"),
              2: nc.dram_tensor("st_out2", [1, 2 * FIN], F32, addr_space="Shared"),
              3: nc.dram_tensor("st_out3", [1, 2 * C], F32, addr_space="Shared")}
    pl_in = nc.dram_tensor("pl_in", [C, G], F32)
    pl_out = nc.dram_tensor("pl_out", [C, G], F32, addr_space="Shared")
    RG = [list(range(P))]

    with tile.TileContext(nc) as tc:
        with tc.tile_pool(name="cn", bufs=1) as cn, \
             tc.tile_pool(name="xb", bufs=1) as xb, \
             tc.tile_pool(name="gp", bufs=3) as gp, \
             tc.tile_pool(name="sp", bufs=2) as sp, \
             tc.tile_pool(name="wp", bufs=2) as wp, \
             tc.tile_pool(name="sm", bufs=2) as sm, \
             tc.tile_pool(name="psA", bufs=2, space="PSUM") as psA, \
             tc.tile_pool(name="psB", bufs=1, space="PSUM") as psB, \
             tc.tile_pool(name="psU", bufs=2, space="PSUM") as psU:

            # ---- load constants ----
            def cload(name, shape, dtype, dram, rearr=None, **kw):
                t = cn.tile(shape, dtype, tag=name)
                src = dram[:] if rearr is None else dram[:].rearrange(rearr, **kw)
                nc.sync.dma_start(t[:], src)
                return t

            idx_sb = cload("idx", [128, NE // 16], I16, gidx_d)
            pool_sb = cload("pool", [128, NT, G], F32, pool_d, "n p g -> p n g")
            ident_sb = cload("ident", [128, 128], F32, ident_d)
            encb_sb = cload("encb", [1, FIN], F32, encb_d)
            eb1_sb = cload("eb1", [1, SS12], F32, eb1_d)
            w3_sb = cload("w3", [128, 4, SS3], BF16, w3_d, "(k p) x -> p k x", p=128)
            linW_sb = cload("linW", [C, NCLS], F32, linW_d)
            linb_sb = cload("linb", [NCLS, 1], F32, linb_d)
            bn_sb = {ly: (cload(f"g{ly}", [1, wd], F32, bn_d[ly][0]),
                          cload(f"be{ly}", [1, wd], F32, bn_d[ly][1]))
                     for ly, wd in ((1, FIN), (2, FIN), (3, C))}
            x0T_sb = xb.tile([128, 4, NT * 128], BF16, tag="lhsT")
            nc.sync.dma_start(x0T_sb[:], x0T_d[:].rearrange("(k p) x -> p k x", p=128))

            ones_c = cn.tile([128, 1], BF16, tag="ones_c")
            nc.vector.memset(ones_c[:], 1.0)
            ones_cf = cn.tile([128, 1], F32, tag="ones_cf")
            nc.vector.memset(ones_cf[:], 1.0)
            ones_r = cn.tile([1, 128], F32, tag="ones_r")
            nc.vector.memset(ones_r[:], 1.0)
            epsbn_t = cn.tile([1, 1], F32, tag="epsbn")
            nc.vector.memset(epsbn_t[:], EPS_BN)

            # big rotating node-feature buffers (f32)
            bufs = [xb.tile([128, NT, FIN], F32, tag=f"big{i}", name=f"big{i}") for i in range(3)]

            def nvalid(n):
                return 128 if n < NT - 1 else LAST

            # ---------- h_ext matmul phase ----------
            def h_phase(ly, lhsT_sb, wcat_sb, wofs, ss, rowv, bias_sb, xe_buf,
                        sdloc):
                for n in range(NT):
                    ht = sm.tile([128, SS12], BF16, tag="hrow")
                    if ly == 1:
                        pxe = psA.tile([128, FIN], F32, tag="mm5")
                        for k in range(4):
                            nc.tensor.matmul(
                                pxe[:], lhsT_sb[:, k, 128 * n:128 * (n + 1)],
                                wcat_sb[:, k, 0:FIN], start=(k == 0), stop=False)
                        nc.tensor.matmul(pxe[:], ones_r[:],
                                         encb_sb[:],
                                         start=False, stop=True,
                                         skip_group_check=True)
                        nc.vector.tensor_copy(xe_buf[:, n, :], pxe[:])
                    p5 = psA.tile([128, FIN], F32, tag="mm5")
                    pS = psB.tile([128, SS3], F32, tag="mmS")
                    nh = ss - FIN if ly < 3 else 2
                    fh = FIN if ly < 3 else C
                    for k in range(4):
                        lt = lhsT_sb[:, k, 128 * n:128 * (n + 1)]
                        if ly < 3:
                            nc.tensor.matmul(p5[:], lt,
                                             wcat_sb[:, k, wofs:wofs + FIN],
                                             start=(k == 0), stop=(k == 3 and ly == 2))
                            nc.tensor.matmul(pS[:, 0:16], lt,
                                             wcat_sb[:, k, wofs + FIN:wofs + ss],
                                             start=(k == 0), stop=(k == 3 and ly == 2))
                        else:
                            nc.tensor.matmul(pS[:, 0:SS3], lt,
                                             wcat_sb[:, k, 0:SS3],
                                             start=(k == 0), stop=(k == 3))
                    if ly == 1:
                        nc.tensor.matmul(p5[:], ones_r[:],
                                         bias_sb[:, 0:FIN],
                                         start=False, stop=True,
                                         skip_group_check=True)
                        nc.tensor.matmul(pS[:, 0:16], ones_r[:],
                                         bias_sb[:, FIN:ss],
                                         start=False, stop=True,
                                         skip_group_check=True)
                    if ly < 3:
                        nc.vector.tensor_copy(ht[:, 0:FIN], p5[:])
                        nc.vector.tensor_copy(ht[:, FIN:ss], pS[:, 0:16])
                        nc.vector.tensor_copy(sdloc[:, n, :],
                                              pS[:, 8:16])
                    else:
                        nc.vector.tensor_copy(ht[:, 0:SS3], pS[:, 0:SS3])
                        nc.vector.tensor_copy(sdloc[:, n, :], pS[:, 65:66])
                    v = nvalid(n)
                    nc.sync.dma_start(
                        cc_in[ly][128 * n:128 * n + v, 0:ss], ht[0:v, 0:ss])

            # ---------- edge aggregation phase ----------
            def edge_phase(ly, rowv, ss, nh, fh, sdloc, ybuf, sacc, sacc2):
                cph = fh // nh
                nc.vector.memset(sacc[:, 0:fh], 0.0)
                nc.vector.memset(sacc2[:, 0:fh], 0.0)
                for b in range(NT):
                    T = T_list[b]
                    s0 = sbase[b]
                    w_t = wp.tile([128, TMAXB, 8], BF16, tag="w_t")
                    first = True
                    pU = psU.tile([128, FIN], F32, tag="U")
                    pZ = psB.tile([128, 8], F32, tag="Z")
                    for c0 in range(0, T, CH):
                        nsl = min(CH, T - c0)
                        sg = s0 + c0
                        g = gp.tile([128, CH, ROW12 if ly < 3 else ROW3],
                                    BF16, tag="g")
                        nc.gpsimd.dma_gather(
                            g[:, 0:nsl, 0:rowv], cc_out[ly][:],
                            idx_sb[:, 8 * sg:8 * (sg + nsl)],
                            num_idxs=nsl * 128, num_idxs_reg=nsl * 128,
                            elem_size=rowv)
                        Ssb = sp.tile([128, CH, 128], BF16, tag="S")
                        STsb = sp.tile([128, CH, 128], BF16, tag="ST")
                        nc.sync.dma_start(
                            Ssb[:, 0:nsl, :],
                            S_d[sg:sg + nsl].rearrange("t p m -> p t m"))
                        nc.sync.dma_start(
                            STsb[:, 0:nsl, :],
                            ST_d[sg:sg + nsl].rearrange("t p m -> p t m"))
                        psd = psB.tile([128, CH * 8], F32, tag="sd", bufs=2)
                        for t in range(nsl):
                            nc.tensor.matmul(
                                psd[:, t * nh:(t + 1) * nh], STsb[:, t, :],
                                sdloc[:, b, :], start=True, stop=True)
                        lg = wp.tile([128, CH * 8], F32, tag="lg")
                        nc.vector.tensor_tensor(
                            out=lg[:, 0:nsl * nh],
                            in0=g[:, 0:nsl, fh:fh + nh],
                            in1=psd[:, 0:nsl * nh], op=AL.add)
                        nc.vector.scalar_tensor_tensor(
                            out=lg[:, 0:nsl * nh], in0=lg[:, 0:nsl * nh],
                            scalar=0.2, in1=lg[:, 0:nsl * nh],
                            op0=AL.mult, op1=AL.max)
                        nc.scalar.activation(
                            w_t[:, c0:c0 + nsl, 0:nh], lg[:, 0:nsl * nh],
                            ACTF.Exp)
                        nc.vector.tensor_tensor(
                            out=g[:, 0:nsl, 0:fh], in0=g[:, 0:nsl, 0:fh],
                            in1=w_t[:, c0:c0 + nsl, 0:nh].unsqueeze(3)
                                .broadcast_to([128, nsl, nh, cph]),
                            op=AL.mult)
                        for t in range(nsl):
                            nc.tensor.matmul(
                                pU[:, 0:fh], Ssb[:, t, :], g[:, t, 0:fh],
                                start=first, stop=(c0 + t == T - 1),
                                skip_group_check=True)
                            nc.tensor.matmul(
                                pZ[:, 0:nh], Ssb[:, t, :],
                                w_t[:, c0 + t, 0:nh],
                                start=first, stop=(c0 + t == T - 1),
                                skip_group_check=True)
                            first = False
                    rz = sm.tile([128, 8], F32, tag="rz")
                    nc.vector.tensor_scalar_add(rz[:, 0:nh], pZ[:, 0:nh], EPS_Z)
                    nc.vector.reciprocal(rz[:, 0:nh], rz[:, 0:nh])
                    nc.vector.tensor_tensor(
                        out=ybuf[:, b, 0:fh], in0=pU[:, 0:fh],
                        in1=rz[:, 0:nh].unsqueeze(2)
                            .broadcast_to([128, nh, cph]),
                        op=AL.mult)
                    y2 = sm.tile([128, FIN], F32, tag="y2")
                    nc.scalar.activation(y2[:, 0:fh], ybuf[:, b, 0:fh],
                                         ACTF.Square)
                    nc.vector.tensor_tensor(out=sacc[:, 0:fh],
                                            in0=sacc[:, 0:fh],
                                            in1=ybuf[:, b, 0:fh], op=AL.add)
                    nc.vector.tensor_tensor(out=sacc2[:, 0:fh],
                                            in0=sacc2[:, 0:fh],
                                            in1=y2[:, 0:fh], op=AL.add)

            # ---------- BN + (ELU + residual) ----------
            def bn_chain(ly, fh, ybuf, xprev, xnext, sacc, sacc2):
                g_sb, be_sb = bn_sb[ly]
                pst1 = psA.tile([1, FIN], F32, tag="mm5")
                pst2 = psA.tile([1, FIN], F32, tag="mm5")
                nc.tensor.matmul(pst1[:, 0:fh], ones_cf[:], sacc[:, 0:fh],
                                 start=True, stop=True)
                nc.tensor.matmul(pst2[:, 0:fh], ones_cf[:], sacc2[:, 0:fh],
                                 start=True, stop=True)
                stat = sm.tile([1, 2 * FIN], F32, tag="stat")
                nc.vector.tensor_copy(stat[:, 0:fh], pst1[:, 0:fh])
                nc.vector.tensor_copy(stat[:, fh:2 * fh], pst2[:, 0:fh])
                nc.sync.dma_start(st_in[ly][:], stat[:, 0:2 * fh])
                nc.gpsimd.collective_compute(
                    "AllReduce", AL.add, replica_groups=RG,
                    ins=[st_in[ly][:]], outs=[st_out[ly][:]])
                st2 = sm.tile([1, 2 * FIN], F32, tag="stat2")
                nc.sync.dma_start(st2[:, 0:2 * fh], st_out[ly][:])
                mu = st2[:, 0:fh]
                ex2 = st2[:, fh:2 * fh]
                nc.vector.tensor_scalar_mul(mu, mu, 1.0 / N)
                nc.vector.tensor_scalar_mul(ex2, ex2, 1.0 / N)
                var = sm.tile([1, FIN], F32, tag="var")
                nc.vector.tensor_tensor(out=var[:, 0:fh], in0=mu, in1=mu,
                                        op=AL.mult)
                nc.vector.tensor_tensor(out=var[:, 0:fh], in0=ex2,
                                        in1=var[:, 0:fh], op=AL.subtract)
                sd = sm.tile([1, FIN], F32, tag="sdv")
                nc.scalar.activation(sd[:, 0:fh], var[:, 0:fh], ACTF.Sqrt,
                                     bias=epsbn_t[:])
                nc.vector.reciprocal(sd[:, 0:fh], sd[:, 0:fh])
                scf = sm.tile([1, FIN], F32, tag="scf")
                nc.vector.tensor_tensor(out=scf[:, 0:fh], in0=g_sb[:],
                                        in1=sd[:, 0:fh], op=AL.mult)
                shf = sm.tile([1, FIN], F32, tag="shf")
                nc.vector.tensor_tensor(out=shf[:, 0:fh], in0=scf[:, 0:fh],
                                        in1=mu, op=AL.mult)
                nc.vector.tensor_tensor(out=shf[:, 0:fh], in0=be_sb[:],
                                        in1=shf[:, 0:fh], op=AL.subtract)
                scT = sm.tile([128, FIN], F32, tag="scT")
                shT = sm.tile([128, FIN], F32, tag="shT")
                nc.gpsimd.partition_broadcast(scT[:, 0:fh], scf[:, 0:fh])
                nc.gpsimd.partition_broadcast(shT[:, 0:fh], shf[:, 0:fh])
                for n in range(NT):
                    v = sm.tile([128, FIN], F32, tag="cht")
                    nc.vector.tensor_tensor(out=v[:, 0:fh],
                                            in0=ybuf[:, n, 0:fh],
                                            in1=scT[:, 0:fh], op=AL.mult)
                    nc.vector.tensor_tensor(out=v[:, 0:fh], in0=v[:, 0:fh],
                                            in1=shT[:, 0:fh], op=AL.add)
                    if ly == 3:
                        nc.vector.tensor_copy(xnext[:, n, 0:fh], v[:, 0:fh])
                        continue
                    m = sm.tile([128, FIN], F32, tag="che")
                    nc.vector.tensor_scalar_min(m[:, 0:fh], v[:, 0:fh], 0.0)
                    nc.scalar.activation(m[:, 0:fh], m[:, 0:fh], ACTF.Exp)
                    nc.vector.scalar_tensor_tensor(
                        out=v[:, 0:fh], in0=v[:, 0:fh], scalar=0.0,
                        in1=m[:, 0:fh], op0=AL.max, op1=AL.add)
                    nc.vector.scalar_tensor_tensor(
                        out=xnext[:, n, 0:fh], in0=v[:, 0:fh], scalar=-1.0,
                        in1=xprev[:, n, 0:fh], op0=AL.add, op1=AL.add)

            # ---------- transpose a -> aT (bf16) ----------
            def transpose_phase(abuf, aT):
                for n in range(NT):
                    for k in range(4):
                        tr = psB.tile([128, 128], F32, tag="sd", bufs=2, name="tr")
                        nc.tensor.transpose(
                            tr[:], abuf[:, n, 128 * k:128 * (k + 1)],
                            ident_sb[:])
                        nc.vector.tensor_copy(
                            aT[:, k, 128 * n:128 * (n + 1)], tr[:])

            # =========== emit program ===========
            xe, ybuf1, a1 = bufs[0], bufs[1], bufs[2]
            sdloc = xb.tile([128, NT, 8], BF16, tag="sdloc")
            wcat0 = cn.tile([128, 4, 1040], BF16, tag="wcat")
            nc.sync.dma_start(wcat0[:], rhs0_d[:].rearrange("(k p) x -> p k x", p=128))

            # encoder + L1 h
            h_phase(1, x0T_sb, wcat0, FIN, SS12, ROW12, eb1_sb, xe, sdloc)
            nc.gpsimd.collective_compute(
                "AllGather", AL.bypass, replica_groups=RG,
                ins=[cc_in[1][:]], outs=[cc_out[1][:]])
            sacc1 = sm.tile([128, FIN], F32, tag="sacc", bufs=1)
            sacc1b = sm.tile([128, FIN], F32, tag="sacc2", bufs=1)
            edge_phase(1, ROW12, SS12, H, FIN, sdloc, ybuf1, sacc1, sacc1b)
            bn_chain(1, FIN, ybuf1, xe, a1, sacc1, sacc1b)

            # L2: a1 -> aT, h, edges (reuse xe buf as ybuf2, ybuf1 as a2)
            aT = xb.tile([128, 4, NT * 128], BF16, tag="lhsT")
            transpose_phase(a1, aT)
            wcat2 = cn.tile([128, 4, SS12], BF16, tag="wcat")
            nc.sync.dma_start(wcat2[:], w2_d[:].rearrange("(k p) x -> p k x", p=128))
            sdloc2 = xb.tile([128, NT, 8], BF16, tag="sdloc")
            h_phase(2, aT, wcat2, 0, SS12, ROW12, None, None, sdloc2)
            nc.gpsimd.collective_compute(
                "AllGather", AL.bypass, replica_groups=RG,
                ins=[cc_in[2][:]], outs=[cc_out[2][:]])
            ybuf2, a2 = xe, ybuf1
            sacc2a = sm.tile([128, FIN], F32, tag="sacc", bufs=1)
            sacc2b = sm.tile([128, FIN], F32, tag="sacc2", bufs=1)
            edge_phase(2, ROW12, SS12, H, FIN, sdloc2, ybuf2, sacc2a, sacc2b)
            bn_chain(2, FIN, ybuf2, a1, a2, sacc2a, sacc2b)

            # L3
            aT2 = xb.tile([128, 4, NT * 128], BF16, tag="lhsT")
            transpose_phase(a2, aT2)
            sdloc3 = xb.tile([128, NT, 1], BF16, tag="sdloc")
            h_phase(3, aT2, w3_sb, 0, SS3, ROW3, None, None, sdloc3)
            nc.gpsimd.collective_compute(
                "AllGather", AL.bypass, replica_groups=RG,
                ins=[cc_in[3][:]], outs=[cc_out[3][:]])
            y3 = xb.tile([128, NT, C], F32, tag="y3")
            x3 = xb.tile([128, NT, C], F32, tag="x3")
            sacc3a = sm.tile([128, FIN], F32, tag="sacc", bufs=1)
            sacc3b = sm.tile([128, FIN], F32, tag="sacc2", bufs=1)
            edge_phase(3, ROW3, SS3, 1, C, sdloc3, y3, sacc3a, sacc3b)
            bn_chain(3, C, y3, None, x3, sacc3a, sacc3b)

            # pooling: xgT[f, g] = sum_n x3[n, f] * pool[n, g]
            pxg = psB.tile([C, G], F32, tag="sd", bufs=2, name="pxg")
            for n in range(NT):
                nc.tensor.matmul(pxg[:], x3[:, n, :], pool_sb[:, n, :],
                                 start=(n == 0), stop=(n == NT - 1))
            xg = sm.tile([C, G], F32, tag="xg")
            nc.vector.tensor_copy(xg[:], pxg[:])
            nc.sync.dma_start(pl_in[:], xg[:])
            nc.gpsimd.collective_compute(
                "AllReduce", AL.add, replica_groups=RG,
                ins=[pl_in[:]], outs=[pl_out[:]])
            xg2 = sm.tile([C, G], F32, tag="xg2")
            nc.sync.dma_start(xg2[:], pl_out[:])
            # outT[nc, g] = linW.T @ xgT  (contract over f)
            pot = psB.tile([NCLS, G], F32, tag="sd", bufs=2, name="pot")
            nc.tensor.matmul(pot[:], linW_sb[:], xg2[:], start=True, stop=True)
            outT = sm.tile([NCLS, G], F32, tag="outT")
            nc.scalar.activation(outT[:], pot[:], ACTF.Identity,
                                 bias=linb_sb[:])
            pfin = psB.tile([G, NCLS], F32, tag="sd", bufs=2, name="pfin")
            nc.tensor.transpose(pfin[:], outT[:], ident_sb[0:NCLS, 0:NCLS])
            fin = sm.tile([G, NCLS], F32, tag="fin_sb")
            nc.vector.tensor_copy(fin[:], pfin[:])
            nc.sync.dma_start(out_d[:], fin[:])

    nc.finalize()
    return nc


_CACHE = {}


def _get_nc(T_key, TT):
    if T_key not in _CACHE:
        _CACHE[T_key] = _build(list(T_key), TT)
    return _CACHE[T_key]


def kernel(**inputs):
    T_list, TT, per_core, shared = _prep(inputs)
    nc = _get_nc(tuple(T_list), TT)
    in_maps = []
    for c in range(P):
        pc = per_core[c]
        m = dict(S=pc['S'], ST=pc['ST'], gidx=pc['gidx'],
                 x0T=pc['x0T'], pool=pc['pool'],
                 rhs0=shared['rhs0'], w2=shared['w2'], w3=shared['w3'],
                 encb=shared['encb'], eb1=shared['eb1'],
                 g1=shared['g1'], be1=shared['be1'],
                 g2=shared['g2'], be2=shared['be2'],
                 g3=shared['g3'], be3=shared['be3'],
                 linW=shared['linW'], linb=shared['linb'],
                 ident=shared['ident'])
        in_maps.append(m)
    res = run_bass_kernel_spmd(nc, in_maps, core_ids=list(range(P)))
    return np.asarray(res.results[0]['out'], np.float32)


def run_traced(**inputs):
    """Like kernel() but returns (output, BassKernelResults with trace)."""
    T_list, TT, per_core, shared = _prep(inputs)
    nc = _get_nc(tuple(T_list), TT)
    in_maps = []
    for c in range(P):
        pc = per_core[c]
        m = dict(S=pc['S'], ST=pc['ST'], gidx=pc['gidx'],
                 x0T=pc['x0T'], pool=pc['pool'], **shared)
        in_maps.append(m)
    res = run_bass_kernel_spmd(nc, in_maps, core_ids=list(range(P)),
                               trace=True)
    return np.asarray(res.results[0]['out'], np.float32), res
